# revision 2
# baseline (speedup 1.0000x reference)
"""MoE layer (dense top-2 routing) on 8 Trainium2 NeuronCores — v3.

Like v2 (host fp32 routing -> device bf16 matmuls for selected experts ->
host scatter-add) but with HALF-TILE balanced scheduling: the unit of work
is a (128-token tile, 512-wide output half).  For this input the per-expert
half-tile counts are [32,30,32,36,36,34,32,32] (sum 264 = 33 per core), and
the schedule is exactly 15 own-expert FULL tiles + 3 helper HALF tiles per
core with zero dummy work — the makespan optimum (PE streaming time is the
wall, so time ~ matmul count: 15*16 + 3*8 = 264 N=512 matmuls per core).

One SPMD program, per-core data.  Slot->weight mapping is position-based:
own slots use the core's own expert weight block; helper half-slot j uses
helper half-block j (an O-half of any overloaded expert's weights).

DMA layout (per core):
  xg  [m1, P, KT*P]     own x tiles: xg[t,p,k*P+j] = score*x[tok(t,j),k*P+p]
  xgh [nh, P, KT*P]     helper x tiles, same layout
  wg  [P, OT*KT*OS]     own weights: wg[p, ot*KT*OS+k*OS+j] = W[k*P+p, ot*OS+j]
  wgh [nh, P, KT*OS]    helper half-blocks: wgh[j,p,k*OS+i] = W_e[k*P+p, h*OS+i]
  out  [m1*P, O]        own outputs
  outh [nh*P, OS]       helper outputs (one O-half each)
"""

import numpy as np

B, S, D, O, E = 4, 2048, 1024, 1024, 8
NCORES = 8
P = 128
KT = D // P          # contraction tiles over D
OS = 512             # psum bank width (fp32)
OT = O // OS
WB = OT * KT * OS    # own weight block free size per partition


def build_nc(m1=15, nh=3, reps=1, hwloop=1):
    import concourse.bacc as bacc
    import concourse.mybir as mybir
    import concourse.tile as tile

    f32 = mybir.dt.float32
    bf16 = mybir.dt.bfloat16

    nc = bacc.Bacc()
    xg_d = nc.declare_dram_parameter("xg", [m1, P, KT * P], bf16,
                                     isOutput=False)
    xgh_d = nc.declare_dram_parameter("xgh", [max(nh, 1), P, KT * P], bf16,
                                      isOutput=False)
    wg_d = nc.declare_dram_parameter("wg", [P, WB], bf16, isOutput=False)
    wgh_d = nc.declare_dram_parameter("wgh", [max(nh, 1), P, KT * OS], bf16,
                                      isOutput=False)
    out_d = nc.declare_dram_parameter("out", [m1 * P, O], bf16,
                                      isOutput=True)
    outh_d = nc.declare_dram_parameter("outh", [max(nh, 1) * P, OS], bf16,
                                       isOutput=True)

    with tile.TileContext(nc) as tc:
        with (
            tc.tile_pool(name="sb", bufs=2) as sb,
            tc.tile_pool(name="ps", bufs=8, space="PSUM") as ps_pool,
        ):
            def load():
                # own x + own w ride the SP ring; helper x/w + out-DMAs ride
                # the ACT ring.
                w0 = sb.tile([P, WB], bf16, tag="w0", name="w0", bufs=2)
                nc.sync.dma_start(out=w0[:], in_=wg_d[:])
                whs = []
                for h in range(nh):
                    wh = sb.tile([P, KT * OS], bf16, tag="wh", name=f"wh{h}",
                                 bufs=2 * max(nh, 1))
                    nc.scalar.dma_start(out=wh[:], in_=wgh_d[h])
                    whs.append(wh)
                xts = []
                for t in range(m1):
                    xt = sb.tile([P, KT * P], bf16, tag="x", name=f"x{t}",
                                 bufs=8)
                    nc.sync.dma_start(out=xt[:], in_=xg_d[t])
                    xts.append(xt)
                xhs = []
                for h in range(nh):
                    xh = sb.tile([P, KT * P], bf16, tag="xh", name=f"xh{h}",
                                 bufs=2 * max(nh, 1))
                    nc.scalar.dma_start(out=xh[:], in_=xgh_d[h])
                    xhs.append(xh)
                return w0, whs, xts, xhs

            def one_rep():
                w0, whs, xts, xhs = load()
                for t in range(m1):
                    acc = sb.tile([P, O], bf16, tag="acc", name="acc",
                                  bufs=4)
                    pss = [ps_pool.tile([P, OS], f32, tag="ps", name="ps")
                           for _ in range(OT)]
                    # k-major: both psum banks accumulate under one
                    # stationary x-slice per k
                    for k in range(KT):
                        for ot in range(OT):
                            nc.tensor.matmul(
                                pss[ot][:],
                                lhsT=xts[t][:, k * P:(k + 1) * P],
                                rhs=w0[:, ot * KT * OS + k * OS:
                                       ot * KT * OS + (k + 1) * OS],
                                start=(k == 0), stop=(k == KT - 1))
                    # evictions on different banks: ACT + DVE in parallel
                    nc.scalar.copy(acc[:, 0:OS], pss[0][:])
                    nc.vector.tensor_copy(out=acc[:, OS:2 * OS],
                                          in_=pss[1][:])
                    nc.scalar.dma_start(out=out_d[t * P:(t + 1) * P, :],
                                        in_=acc[:])
                for h in range(nh):
                    acch = sb.tile([P, OS], bf16, tag="acch", name="acch",
                                   bufs=2 * max(nh, 1))
                    ph = ps_pool.tile([P, OS], f32, tag="ps", name="ps")
                    for k in range(KT):
                        nc.tensor.matmul(
                            ph[:],
                            lhsT=xhs[h][:, k * P:(k + 1) * P],
                            rhs=whs[h][:, k * OS:(k + 1) * OS],
                            start=(k == 0), stop=(k == KT - 1))
                    if h % 2 == 0:
                        nc.scalar.copy(acch[:], ph[:])
                    else:
                        nc.vector.tensor_copy(out=acch[:], in_=ph[:])
                    nc.scalar.dma_start(out=outh_d[h * P:(h + 1) * P, :],
                                        in_=acch[:])

            if hwloop > 1:
                with tc.For_i(0, hwloop, 1, hint_engines=(
                        mybir.EngineType.PE, mybir.EngineType.Activation,
                        mybir.EngineType.SP, mybir.EngineType.DVE)):
                    for _rep in range(reps):
                        one_rep()
            else:
                for _rep in range(reps):
                    one_rep()

    nc.compile()
    return nc


_cache = {}


def _get_nc(m1, nh):
    if (m1, nh) not in _cache:
        _cache[(m1, nh)] = build_nc(m1=m1, nh=nh)
    return _cache[(m1, nh)]


def route(x, gate_w, gate_b):
    """Host top-2 routing in fp32: returns (x2, e1, e2, p1, p2) per token."""
    x2 = np.asarray(x, np.float32).reshape(B * S, D)
    logits = x2 @ np.asarray(gate_w, np.float32).T + np.asarray(
        gate_b, np.float32)
    e1 = logits.argmax(-1)
    rows = np.arange(len(e1))
    l1 = logits[rows, e1]
    masked = logits.copy()
    masked[rows, e1] = -np.inf
    e2 = masked.argmax(-1)
    l2 = masked[rows, e2]
    p2 = 1.0 / (1.0 + np.exp(l1 - l2))
    p1 = 1.0 - p2
    return x2, e1, e2, p1, p2


def plan_slots(counts):
    """Half-tile balanced schedule.

    Own slots: m1 full tiles of the core's own expert (m1 = min tile count
    so no own slot is ever a dummy).  Helper slots: nh half-tiles; all
    half-tiles beyond 2*m1 of any expert round-robin into them."""
    nt_e = [max(1, -(-c // P)) for c in counts]
    m1 = min(nt_e)
    while True:
        leftover = sum(max(0, 2 * n - 2 * m1) for n in nt_e)
        nh = -(-leftover // NCORES)
        # helper positions must cover the leftovers; they always do by
        # construction, but keep total slots >= work if m1 was too large
        if leftover <= nh * NCORES:
            break
        m1 -= 1
    # helper job list: (expert, tile, half) for tiles >= m1
    jobs = [(e, t, h) for e in range(E) for t in range(m1, nt_e[e])
            for h in range(2)]
    helper = [[None] * nh for _ in range(NCORES)]
    positions = [(c, h) for h in range(nh) for c in range(NCORES)]
    for job, (c, h) in zip(jobs, positions):
        helper[c][h] = job
    return m1, nh, nt_e, helper


def make_in_maps(x, gate_w, gate_b, expert_w, expert_b):
    import ml_dtypes

    x2, e1, e2, p1, p2 = route(x, gate_w, gate_b)
    idxs, scores = [], []
    for e in range(E):
        i1 = np.nonzero(e1 == e)[0]
        i2 = np.nonzero(e2 == e)[0]
        idxs.append(np.concatenate([i1, i2]))
        scores.append(np.concatenate([p1[i1], p2[i2]]).astype(np.float32))
    counts = [len(i) for i in idxs]
    m1, nh, nt_e, helper = plan_slots(counts)

    wblocks = {}
    for e in range(E):
        w = np.asarray(expert_w[e], np.float32)
        wblocks[e] = np.ascontiguousarray(
            w.reshape(KT, P, OT, OS).transpose(1, 2, 0, 3)
        ).reshape(P, WB).astype(ml_dtypes.bfloat16)

    def xtile(e, t):
        lo, hi = t * P, min((t + 1) * P, counts[e])
        xx = np.zeros((P, D), np.float32)
        xx[:hi - lo] = x2[idxs[e][lo:hi]] * scores[e][lo:hi, None]
        return xx.reshape(P, KT, P).transpose(2, 1, 0).reshape(
            P, KT * P).astype(ml_dtypes.bfloat16)

    in_maps = []
    for c in range(NCORES):
        xg = np.zeros((m1, P, KT * P), ml_dtypes.bfloat16)
        for t in range(min(nt_e[c], m1)):
            xg[t] = xtile(c, t)
        xgh = np.zeros((max(nh, 1), P, KT * P), ml_dtypes.bfloat16)
        wgh = np.zeros((max(nh, 1), P, KT * OS), ml_dtypes.bfloat16)
        for h in range(nh):
            job = helper[c][h]
            if job is None:
                continue
            e, t, hf = job
            xgh[h] = xtile(e, t)
            wgh[h] = wblocks[e][:, hf * KT * OS:(hf + 1) * KT * OS]
        in_maps.append({"xg": xg, "xgh": xgh, "wg": wblocks[c],
                        "wgh": np.ascontiguousarray(wgh)})
    return in_maps, (idxs, counts, m1, nh, nt_e, helper), (e1, e2, p1, p2)


def scatter_out(outs, ouths, idxs, counts, m1, nh, nt_e, helper):
    out = np.zeros((B * S, O), np.float32)
    for c in range(NCORES):
        for t in range(min(nt_e[c], m1)):
            lo, hi = t * P, min((t + 1) * P, counts[c])
            out[idxs[c][lo:hi]] += outs[c][t * P: t * P + hi - lo].astype(
                np.float32)
        for h in range(nh):
            job = helper[c][h]
            if job is None:
                continue
            e, t, hf = job
            lo, hi = t * P, min((t + 1) * P, counts[e])
            out[idxs[e][lo:hi], hf * OS:(hf + 1) * OS] += \
                ouths[c][h * P: h * P + hi - lo].astype(np.float32)
    return out


def _spot_check(out, x2, e1, e2, p1, p2, expert_w, expert_b):
    """Exact fp32 recompute of a few sampled token rows; guards against
    transient device/transport garbage (observed rarely under axon).
    bf16 kernel rows sit ~3e-3 from the fp32 value; 0.05 flags only
    catastrophic corruption."""
    ew = np.asarray(expert_w, np.float32)
    eb = np.asarray(expert_b, np.float32)
    toks = np.linspace(0, len(x2) - 1, 16).astype(np.int64)
    for t in toks:
        y = (p1[t] * (x2[t] @ ew[e1[t]] + eb[e1[t]]) +
             p2[t] * (x2[t] @ ew[e2[t]] + eb[e2[t]]))
        rel = np.linalg.norm(out[t] - y) / max(np.linalg.norm(y), 1e-9)
        if rel > 0.05:
            return False
    return True


def kernel(x, gate_w, gate_b, expert_w, expert_b):
    from concourse.bass_utils import run_bass_kernel_spmd

    in_maps, plan, (e1, e2, p1, p2) = make_in_maps(
        x, gate_w, gate_b, expert_w, expert_b)
    idxs, counts, m1, nh, nt_e, helper = plan
    nc = _get_nc(m1, nh)
    x2 = np.asarray(x, np.float32).reshape(B * S, D)
    for _attempt in range(3):
        res = run_bass_kernel_spmd(nc, in_maps, list(range(NCORES)))
        out = scatter_out([r["out"] for r in res.results],
                          [r["outh"] for r in res.results],
                          idxs, counts, m1, nh, nt_e, helper)
        eb = np.asarray(expert_b, np.float32)
        out += p1[:, None].astype(np.float32) * eb[e1]
        out += p2[:, None].astype(np.float32) * eb[e2]
        if _spot_check(out, x2, e1, e2, p1, p2, expert_w, expert_b):
            break
    return out.reshape(B, S, O)
